# revision 1
# baseline (speedup 1.0000x reference)
"""Causal multi-head attention (B=1, S=4096, H=16, D=128) on 8 TRN2 NeuronCores.

Sharding: pure head-parallel SPMD - 16 heads / 8 cores = 2 heads per core.
Each core receives its heads' Q^T, K^T (pre-transposed to [D, S] on host) and
V ([S, D]), and computes full causal attention for those heads. No collectives.

Per-core kernel layout ("layout A" - scores computed transposed):
  S^T[t, q] chunk = matmul(lhsT=K^T[:, tchunk], rhs=Q^T[:, qblock])  (PE)
  P^T = exp(S^T * 1/sqrt(D))  PSUM -> SBUF fp16                      (ACT)
  causal mask fix-up on diagonal chunks (tri-mask mul + memset)      (DVE/GPSIMD)
  out^T[d, q] += matmul(lhsT=V[tchunk], rhs=P^T chunk)  accum PSUM   (PE)
  l[q] = ones^T @ (pairwise-tree-sum of P^T chunks)                  (DVE tree + PE)
  out = out^T * 1/l                                                  (DVE)
Host reassembles [B, S, H, D] from per-core out^T [HPC, D, S].
"""
import math
import os
import sys

for _p in ("/opt/trn_rl_repo", "/root/.axon_site/_ro/trn_rl_repo"):
    if os.path.isdir(_p) and _p not in sys.path:
        sys.path.insert(0, _p)

import numpy as np

import concourse.bass as bass  # noqa: E402
import concourse.mybir as mybir  # noqa: E402
import concourse.tile as tile  # noqa: E402
from concourse import bacc  # noqa: E402
from concourse.bass_utils import run_bass_kernel_spmd  # noqa: E402
from concourse.masks import make_upper_triangular  # noqa: E402

N_CORES = 8
CH = 128  # key/t chunk (PE contraction width)
QB = 512  # query block (PE moving width / PSUM bank)
GRP = 3   # score chunks per ACT group (3 PSUM banks)

F16 = mybir.dt.float16
F32 = mybir.dt.float32


def build(S=4096, HPC=2, qk_dt=F16, pv_dt=F16):
    """Build + compile the per-core Bass program (identical on all cores)."""
    NQ = S // QB
    NCH = S // CH
    RPB = QB // CH  # chunks per q-block row of the diagonal (4)
    np_qk = mybir.dt.np(qk_dt)
    np_pv = mybir.dt.np(pv_dt)

    nc = bacc.Bacc("TRN2", target_bir_lowering=False, debug=False,
                   num_devices=N_CORES)
    qT_d = nc.declare_dram_parameter("qT", [HPC, 128, S], qk_dt, isOutput=False)
    kT_d = nc.declare_dram_parameter("kT", [HPC, 128, S], qk_dt, isOutput=False)
    v_d = nc.declare_dram_parameter("v", [HPC, S, 128], pv_dt, isOutput=False)
    o_d = nc.declare_dram_parameter("outT", [HPC, 128, S], F32, isOutput=True)

    scale = 1.0 / math.sqrt(128.0)

    with tile.TileContext(nc) as tc:
        with (
            tc.tile_pool(name="const", bufs=1) as constp,
            tc.tile_pool(name="kv", bufs=1) as kvp,
            tc.tile_pool(name="qs", bufs=3) as qsp,
            tc.tile_pool(name="panel", bufs=2) as panelp,
            tc.tile_pool(name="tree", bufs=2) as treep,
            tc.tile_pool(name="aux", bufs=2) as auxp,
            tc.tile_pool(name="outp", bufs=3) as outpp,
            tc.tile_pool(name="ps_sc", bufs=2, space="PSUM") as ps_sc,
            tc.tile_pool(name="ps_pv", bufs=1, space="PSUM") as ps_pv,
            tc.tile_pool(name="ps_l", bufs=1, space="PSUM") as ps_l,
        ):
            ones_sb = constp.tile([128, 128], pv_dt, tag="ones")
            nc.gpsimd.memset(ones_sb[:], 1.0)
            tri = constp.tile([128, 128], pv_dt, tag="tri")
            make_upper_triangular(nc, tri[:], val=1.0, diag=True)

            kT_sb = []
            v_sb = []
            for h in range(HPC):
                kt = kvp.tile([128, S], qk_dt, tag=f"kT{h}")
                nc.sync.dma_start(kt[:], kT_d.ap()[h])
                kT_sb.append(kt)
                vt = kvp.tile([128, NCH, 128], pv_dt, tag=f"v{h}")
                nc.sync.dma_start(
                    vt[:], v_d.ap()[h].rearrange("(c p) d -> p c d", p=128))
                v_sb.append(vt)

            def emit_pv_ones(st):
                h, C, pan, tb = st["h"], st["C"], st["pan"], st["tb"]
                pvp = ps_pv.tile([128, QB], F32, tag="pv")
                for c in range(C):
                    nc.tensor.matmul(
                        pvp[:], v_sb[h][:, c, :], pan[:, c * QB:(c + 1) * QB],
                        start=(c == 0), stop=(c == C - 1))
                lp = ps_l.tile([128, QB], F32, tag="l")
                nc.tensor.matmul(lp[:], ones_sb[:], tb[:, :QB],
                                 start=True, stop=True)
                st["pvp"], st["lp"] = pvp, lp

            def emit_finish(st):
                h, qi = st["h"], st["qi"]
                linv = auxp.tile([128, QB], F32, tag="linv")
                nc.vector.reciprocal(linv[:], st["lp"][:])
                ot = outpp.tile([128, QB], F32, tag="ot")
                nc.vector.tensor_mul(ot[:], st["pvp"][:], linv[:])
                nc.sync.dma_start(o_d.ap()[h][:, qi * QB:(qi + 1) * QB], ot[:])

            prev = None
            for h in range(HPC):
                for qi in range(NQ):
                    C = (qi + 1) * RPB  # causal chunk count for this q block
                    qsl = qsp.tile([128, QB], qk_dt, tag="qsl")
                    nc.sync.dma_start(
                        qsl[:], qT_d.ap()[h][:, qi * QB:(qi + 1) * QB])
                    pan = panelp.tile([128, NCH * QB], pv_dt, tag="panel")
                    # QK chunks -> PSUM groups -> exp -> panel
                    for g0 in range(0, C, GRP):
                        n = min(GRP, C - g0)
                        sc = ps_sc.tile([128, GRP * QB], F32, tag="sc")
                        for j in range(n):
                            c = g0 + j
                            nc.tensor.matmul(
                                sc[:, j * QB:(j + 1) * QB],
                                kT_sb[h][:, c * CH:(c + 1) * CH], qsl[:],
                                start=True, stop=True)
                        nc.scalar.activation(
                            pan[:, g0 * QB:(g0 + n) * QB], sc[:, :n * QB],
                            mybir.ActivationFunctionType.Exp, scale=scale)
                    # causal fix-up on the RPB diagonal chunks
                    for k in range(RPB):
                        off = (C - RPB + k) * QB
                        if k > 0:
                            nc.gpsimd.memset(pan[:, off:off + k * CH], 0.0)
                        sl = pan[:, off + k * CH:off + (k + 1) * CH]
                        nc.vector.tensor_mul(sl, sl, tri[:])
                    # deferred tail of the previous q block (keeps PE dense)
                    if prev is not None:
                        emit_pv_ones(prev)
                        emit_finish(prev)
                    # pairwise tree-sum of panel chunks -> tb[:, :QB]
                    tb = treep.tile([128, (NCH // 2) * QB], pv_dt, tag="tree")
                    H0 = C // 2
                    nc.vector.tensor_add(
                        tb[:, :H0 * QB], pan[:, :H0 * QB],
                        pan[:, H0 * QB:C * QB])
                    W = H0
                    while W > 1:
                        Hh = W // 2
                        nc.vector.tensor_add(
                            tb[:, :Hh * QB], tb[:, :Hh * QB],
                            tb[:, (W - Hh) * QB:W * QB])
                        W = W - Hh
                    prev = dict(h=h, qi=qi, C=C, pan=pan, tb=tb)
            emit_pv_ones(prev)
            emit_finish(prev)

    nc.compile()
    return nc, np_qk, np_pv


_CACHE = {}


def _get(S, HPC):
    key = (S, HPC)
    if key not in _CACHE:
        _CACHE[key] = build(S, HPC)
    return _CACHE[key]


def kernel(query, key, value):
    q = np.asarray(query)
    k = np.asarray(key)
    v = np.asarray(value)
    B, S, H, D = q.shape
    assert B == 1 and D == 128 and H % N_CORES == 0
    HPC = H // N_CORES
    nc, np_qk, np_pv = _get(S, HPC)

    in_maps = []
    for c in range(N_CORES):
        hh = slice(c * HPC, (c + 1) * HPC)
        qT = np.ascontiguousarray(
            q[0, :, hh, :].astype(np_qk).transpose(1, 2, 0))
        kT = np.ascontiguousarray(
            k[0, :, hh, :].astype(np_qk).transpose(1, 2, 0))
        vv = np.ascontiguousarray(
            v[0, :, hh, :].astype(np_pv).transpose(1, 0, 2))
        in_maps.append({"qT": qT, "kT": kT, "v": vv})

    res = run_bass_kernel_spmd(nc, in_maps, list(range(N_CORES)))

    out = np.empty((B, S, H, D), np.float32)
    for c in range(N_CORES):
        oT = res.results[c]["outT"]  # [HPC, 128, S] fp32
        out[0, :, c * HPC:(c + 1) * HPC, :] = oT.transpose(2, 0, 1)
    return out.astype(query.dtype)
